# revision 10
# baseline (speedup 1.0000x reference)
"""Bass/Tile Trainium2 kernel for a GPT-2-style transformer block.

Contract: kernel(**inputs) takes the FULL unsharded inputs (B=8, T=1024,
C=768) and returns the FULL [8, 1024, 768] float32 output. Internally the
batch is sharded one-sequence-per-core across 8 NeuronCores (pure data
parallel, no collectives).

All GEMMs run in bf16 on the PE array (fp32 PSUM accumulation); weights are
cast/re-laid-out host-side, activations are cast during PSUM eviction.
LayerNorm statistics, softmax accumulation, and residual adds stay fp32.

Self-contained: all shapes/flags hardcoded for this problem.
"""
import sys

sys.path.insert(0, "/opt/trn_rl_repo")

from contextlib import ExitStack

import numpy as np

import concourse.bacc as bacc
import concourse.bass as bass
import concourse.mybir as mybir
import concourse.tile as tile
from concourse.masks import make_identity

P = 128
T = 1024
C = 768
H = 12
DH = 64
DI = 3072
TT = T // P    # 8 token tiles
KC = C // P    # 6 C-contraction tiles
KI = DI // P   # 24 D_INNER-contraction tiles
MI = DI // P   # 24 inner-feature tiles
LN_EPS = 1e-5

F32 = mybir.dt.float32
BF16 = mybir.dt.bfloat16
AF = mybir.ActivationFunctionType


def build_program(flags):
    """Build the single-core Bass program. flags: dict of bools
    (ln1_wb, ln2_wb, b_attn, b_proj, b_fc, b_out) controlling whether the
    corresponding affine params are applied (skipped when identity/zero)."""
    nc = bacc.Bacc(None, target_bir_lowering=False, debug=False)

    x_d = nc.declare_dram_parameter("x", [T, C], F32, isOutput=False)
    # host-re-laid-out bf16 weights (see _prep_shared below)
    wqk_d = nc.declare_dram_parameter("wqk", [12, P, KC * P], BF16, isOutput=False)
    wv_d = nc.declare_dram_parameter("wv", [P, KC * C], BF16, isOutput=False)
    wproj_d = nc.declare_dram_parameter("wproj", [P, KC * C], BF16, isOutput=False)
    wfc_d = nc.declare_dram_parameter("wfc", [MI, P, KC * P], BF16, isOutput=False)
    wout_d = nc.declare_dram_parameter("wout", [KI, P, C], BF16, isOutput=False)
    opt = {}
    if flags["ln1_wb"]:
        opt["ln1_w"] = nc.declare_dram_parameter("ln1_w", [C], F32, isOutput=False)
        opt["ln1_b"] = nc.declare_dram_parameter("ln1_b", [C], F32, isOutput=False)
    if flags["ln2_wb"]:
        opt["ln2_w"] = nc.declare_dram_parameter("ln2_w", [C], F32, isOutput=False)
        opt["ln2_b"] = nc.declare_dram_parameter("ln2_b", [C], F32, isOutput=False)
    if flags["b_attn"]:
        opt["b_attn"] = nc.declare_dram_parameter("b_attn", [3 * C], F32, isOutput=False)
    if flags["b_proj"]:
        opt["b_proj"] = nc.declare_dram_parameter("b_proj", [C], F32, isOutput=False)
    if flags["b_fc"]:
        opt["b_fc"] = nc.declare_dram_parameter("b_fc", [DI], F32, isOutput=False)
    if flags["b_out"]:
        opt["b_out"] = nc.declare_dram_parameter("b_out", [C], F32, isOutput=False)
    out_d = nc.declare_dram_parameter("out", [T, C], F32, isOutput=True)

    with tile.TileContext(nc) as tc, ExitStack() as ctx:
        # ---------------- resident pool ----------------
        res = ctx.enter_context(tc.tile_pool(name="res", bufs=1))
        x_sb = res.tile([P, TT, C], F32)          # 24 KB/p
        znT = res.tile([P, KC, T], BF16)          # 12 KB/p
        ident = res.tile([P, P], BF16)
        ones_sb = res.tile([P, DH], BF16)
        eps_sb = res.tile([P, 1], F32)

        for n in range(TT):
            nc.sync.dma_start(
                x_sb[:, n, :], x_d.rearrange("(n p) c -> n p c", p=P)[n]
            )
        make_identity(nc, ident)
        nc.vector.memset(ones_sb[:], 1.0)
        nc.vector.memset(eps_sb[:], LN_EPS)

        # broadcast/bias constants
        ln1w_bc = ln1b_bc = ln2w_bc = ln2b_bc = None
        b_qk_pm = b_v_bc = b_proj_bc = b_fc_pm = b_out_bc = None
        consts = ctx.enter_context(tc.tile_pool(name="consts", bufs=1))
        if flags["ln1_wb"]:
            ln1w_bc = consts.tile([P, C], F32)
            ln1b_bc = consts.tile([P, C], F32)
            nc.sync.dma_start(ln1w_bc[:], opt["ln1_w"][None, :].to_broadcast([P, C]))
            nc.sync.dma_start(ln1b_bc[:], opt["ln1_b"][None, :].to_broadcast([P, C]))
        if flags["ln2_wb"]:
            ln2w_bc = consts.tile([P, C], F32)
            ln2b_bc = consts.tile([P, C], F32)
            nc.sync.dma_start(ln2w_bc[:], opt["ln2_w"][None, :].to_broadcast([P, C]))
            nc.sync.dma_start(ln2b_bc[:], opt["ln2_b"][None, :].to_broadcast([P, C]))
        if flags["b_attn"]:
            b_qk_pm = consts.tile([P, 12], F32)
            nc.sync.dma_start(
                b_qk_pm[:], opt["b_attn"][0 : 2 * C].rearrange("(m p) -> p m", p=P)
            )
            b_v_bc = consts.tile([P, C], F32)
            nc.sync.dma_start(
                b_v_bc[:], opt["b_attn"][2 * C : 3 * C][None, :].to_broadcast([P, C])
            )
        if flags["b_proj"]:
            b_proj_bc = consts.tile([P, C], F32)
            nc.sync.dma_start(b_proj_bc[:], opt["b_proj"][None, :].to_broadcast([P, C]))
        if flags["b_fc"]:
            b_fc_pm = consts.tile([P, MI], F32)
            nc.sync.dma_start(b_fc_pm[:], opt["b_fc"].rearrange("(m p) -> p m", p=P))
        if flags["b_out"]:
            b_out_bc = consts.tile([P, C], F32)
            nc.sync.dma_start(b_out_bc[:], opt["b_out"][None, :].to_broadcast([P, C]))

        # ---------- helper: layernorm of one [P, C] tile (bf16 output) ----------
        def layer_norm_tile(pool, src_ap, dst_ap, w_bc, b_bc):
            # bn_stats free-dim max is 512; use 3 subgroups of 256
            SG = 256
            nsg = C // SG
            stats = pool.tile([P, nsg, 6], F32, tag="ln_stats", name="ln_stats")
            for s in range(nsg):
                nc.vector.bn_stats(stats[:, s, :], src_ap[:, s * SG : (s + 1) * SG])
            mv = pool.tile([P, 2], F32, tag="ln_mv", name="ln_mv")
            nc.vector.bn_aggr(mv[:], stats[:])
            rstd = pool.tile([P, 1], F32, tag="ln_rstd", name="ln_rstd")
            nc.scalar.activation(rstd[:], mv[:, 1:2], AF.Sqrt, bias=eps_sb[:])
            nc.vector.reciprocal(rstd[:], rstd[:])
            nmr = pool.tile([P, 1], F32, tag="ln_nmr", name="ln_nmr")
            nc.vector.tensor_mul(nmr[:], mv[:, 0:1], rstd[:])
            nc.vector.tensor_scalar_mul(nmr[:], nmr[:], -1.0)
            if w_bc is None and b_bc is None:
                nc.scalar.activation(
                    dst_ap, src_ap, AF.Identity, bias=nmr[:], scale=rstd[:]
                )
            else:
                tmp = pool.tile([P, C], F32, tag="ln_tmp", name="ln_tmp")
                nc.scalar.activation(
                    tmp[:], src_ap, AF.Identity, bias=nmr[:], scale=rstd[:]
                )
                if w_bc is not None:
                    nc.vector.tensor_mul(tmp[:], tmp[:], w_bc[:])
                if b_bc is not None:
                    nc.vector.tensor_add(tmp[:], tmp[:], b_bc[:])
                nc.any.tensor_copy(dst_ap, tmp[:])

        # ============ scope A: attention half ============
        with ExitStack() as ctxA:
            yT_pool = ctxA.enter_context(tc.tile_pool(name="yT", bufs=1))
            yT = yT_pool.tile([P, KC, T], BF16)  # 12 KB/p

            with ExitStack() as ctx2:
                qkv_pool = ctx2.enter_context(tc.tile_pool(name="qkv", bufs=1))
                q_sb = qkv_pool.tile([P, KC, T], BF16)          # 12 KB/p
                k_sb = qkv_pool.tile([P, KC, T], BF16)          # 12 KB/p
                v_aug = qkv_pool.tile([P, TT, H, DH + 1], BF16)  # 12.2 KB/p
                nc.vector.memset(v_aug[:, :, :, DH : DH + 1], 1.0)

                # ---- phase 1+2: LN1, transpose, qkv GEMMs ----
                with ExitStack() as ctx1:
                    xnT_pool = ctx1.enter_context(tc.tile_pool(name="xnT", bufs=1))
                    xnT = xnT_pool.tile([P, KC, T], BF16)  # 12 KB/p
                    ln_pool = ctx1.enter_context(tc.tile_pool(name="ln1", bufs=3))
                    tp_psum = ctx1.enter_context(
                        tc.tile_pool(name="tp_ps", bufs=2, space="PSUM")
                    )
                    for n in range(TT):
                        xn_t = ln_pool.tile([P, C], BF16, tag="xn", name="xn")
                        layer_norm_tile(ln_pool, x_sb[:, n, :], xn_t[:], ln1w_bc, ln1b_bc)
                        for k in range(KC):
                            pt = tp_psum.tile([P, P], BF16, tag="tp", name="tp")
                            nc.tensor.transpose(
                                pt[:], xn_t[:, k * P : (k + 1) * P], ident[:]
                            )
                            nc.any.tensor_copy(xnT[:, k, n * P : (n + 1) * P], pt[:])

                    # q/k GEMM (transposed out, bf16)
                    wq_pool = ctx1.enter_context(tc.tile_pool(name="wq", bufs=3))
                    gps = ctx1.enter_context(
                        tc.tile_pool(name="gemm_ps", bufs=3, space="PSUM")
                    )
                    for m in range(12):
                        wcol = wq_pool.tile([P, KC, P], BF16, tag="wcol", name="wcol")
                        nc.sync.dma_start(
                            wcol[:], wqk_d[m].rearrange("p (kt j) -> p kt j", j=P)
                        )
                        dst = q_sb if m < KC else k_sb
                        mi = m if m < KC else m - KC
                        for nch in range(2):
                            ps = gps.tile([P, 512], F32, tag="g_ps", name="g_ps")
                            for k in range(KC):
                                nc.tensor.matmul(
                                    ps[:],
                                    wcol[:, k, :],
                                    xnT[:, k, nch * 512 : (nch + 1) * 512],
                                    start=(k == 0),
                                    stop=(k == KC - 1),
                                )
                            bias = b_qk_pm[:, m : m + 1] if flags["b_attn"] else 0.0
                            nc.scalar.activation(
                                dst[:, mi, nch * 512 : (nch + 1) * 512],
                                ps[:],
                                AF.Identity,
                                bias=bias,
                            )

                    # v GEMM (token-major into v_aug)
                    wv_pool = ctx1.enter_context(tc.tile_pool(name="wvp", bufs=1))
                    wv = wv_pool.tile([P, KC, C], BF16)  # 9 KB/p
                    nc.sync.dma_start(
                        wv[:], wv_d.rearrange("p (kt j) -> p kt j", j=C)
                    )
                    for n in range(TT):
                        for c0, cw in ((0, 512), (512, 256)):
                            ps = gps.tile([P, 512], F32, tag="g_ps", name="g_psv")
                            for k in range(KC):
                                nc.tensor.matmul(
                                    ps[:, :cw],
                                    xnT[:, k, n * P : (n + 1) * P],
                                    wv[:, k, c0 : c0 + cw],
                                    start=(k == 0),
                                    stop=(k == KC - 1),
                                )
                            dst = v_aug[:, n, c0 // DH : (c0 + cw) // DH, 0:DH]
                            src = ps[:, :cw].rearrange("p (h d) -> p h d", d=DH)
                            if flags["b_attn"]:
                                nc.vector.tensor_add(
                                    dst,
                                    src,
                                    b_v_bc[:, c0 : c0 + cw].rearrange(
                                        "p (h d) -> p h d", d=DH
                                    ),
                                )
                            else:
                                nc.any.tensor_copy(dst, src)

                # ---- phase 3: attention per head ----
                with ExitStack() as ctx3:
                    sp = ctx3.enter_context(tc.tile_pool(name="attn_sb", bufs=4))
                    ps_s = ctx3.enter_context(
                        tc.tile_pool(name="ps_s", bufs=2, space="PSUM")
                    )
                    ps_y = ctx3.enter_context(
                        tc.tile_pool(name="ps_y", bufs=4, space="PSUM")
                    )
                    ps_b = ctx3.enter_context(
                        tc.tile_pool(name="ps_b", bufs=2, space="PSUM")
                    )
                    for h in range(H):
                        base = (h % 2) * DH
                        mt = h // 2
                        py = [
                            ps_y.tile([P, 512], F32, tag="py", name=f"py{h}_{qc}")
                            for qc in range(2)
                        ]
                        for kt in range(TT):
                            q0 = kt * P
                            pieces = []
                            if q0 < 512:
                                pieces.append((q0, 512 - q0))
                            pieces.append((max(512, q0), T - max(512, q0)))
                            for p0, pw in pieces:
                                st = ps_s.tile([P, 512], F32, tag="st", name="st")
                                nc.tensor.matmul(
                                    st[:, :pw],
                                    k_sb[base : base + DH, mt, q0 : q0 + P],
                                    q_sb[base : base + DH, mt, p0 : p0 + pw],
                                    start=True,
                                    stop=True,
                                )
                                es = sp.tile([P, 512], BF16, tag="es", name="es")
                                nc.scalar.activation(
                                    es[:, :pw], st[:, :pw], AF.Exp, scale=0.125
                                )
                                # causal mask on diagonal part: S^T[k, q],
                                # valid iff q >= k ; q = p0+f, k = q0+p
                                dw = q0 + P - p0
                                if dw > 0:
                                    dw = min(dw, pw)
                                    nc.gpsimd.affine_select(
                                        es[:, :dw],
                                        es[:, :dw],
                                        pattern=[[1, dw]],
                                        compare_op=mybir.AluOpType.is_ge,
                                        fill=0.0,
                                        base=p0 - q0,
                                        channel_multiplier=-1,
                                    )
                                qc = 0 if p0 < 512 else 1
                                nc.tensor.matmul(
                                    py[qc][
                                        0 : DH + 1, p0 - qc * 512 : p0 - qc * 512 + pw
                                    ],
                                    v_aug[:, kt, h, :],
                                    es[:, :pw],
                                    start=(kt == 0),
                                    stop=(kt == TT - 1 or (qc == 0 and kt == 3)),
                                )
                        for qc in range(2):
                            rp = sp.tile([P, 512], BF16, tag="rp", name="rp")
                            with nc.allow_low_precision(
                                reason="softmax recip feeds bf16 bcast matmul"
                            ):
                                nc.vector.reciprocal(
                                    rp[DH : DH + 1, :], py[qc][DH : DH + 1, :]
                                )
                            pb = ps_b.tile([DH, 512], F32, tag="pb", name="pb")
                            nc.tensor.matmul(
                                pb[:],
                                ones_sb[DH : DH + 1, :],
                                rp[DH : DH + 1, :],
                                start=True,
                                stop=True,
                            )
                            pbs = sp.tile([DH, 512], F32, tag="pbs", name="pbs")
                            nc.any.tensor_copy(pbs[:], pb[:])
                            yn = sp.tile([DH, 512], BF16, tag="yn", name="yn")
                            nc.vector.tensor_mul(yn[:], py[qc][0:DH, :], pbs[:])
                            nc.sync.dma_start(
                                yT[base : base + DH, mt, qc * 512 : (qc + 1) * 512],
                                yn[:],
                            )

            # ---- phase 4: proj GEMM + residual -> z; LN2 -> znT ----
            with ExitStack() as ctx4:
                wproj_pool = ctx4.enter_context(tc.tile_pool(name="wprojp", bufs=1))
                w_proj_sb = wproj_pool.tile([P, KC, C], BF16)  # 9 KB/p
                nc.sync.dma_start(
                    w_proj_sb[:], wproj_d.rearrange("p (kt j) -> p kt j", j=C)
                )
                zp = ctx4.enter_context(tc.tile_pool(name="proj", bufs=3))
                zps = ctx4.enter_context(
                    tc.tile_pool(name="proj_ps", bufs=3, space="PSUM")
                )
                tp_psum2 = ctx4.enter_context(
                    tc.tile_pool(name="tp_ps2", bufs=2, space="PSUM")
                )
                for n in range(TT):
                    z_t = zp.tile([P, C], F32, tag="z", name="z")
                    for c0, cw in ((0, 512), (512, 256)):
                        ps = zps.tile([P, 512], F32, tag="z_ps", name="z_ps")
                        for k in range(KC):
                            nc.tensor.matmul(
                                ps[:, :cw],
                                yT[:, k, n * P : (n + 1) * P],
                                w_proj_sb[:, k, c0 : c0 + cw],
                                start=(k == 0),
                                stop=(k == KC - 1),
                            )
                        if flags["b_proj"]:
                            nc.vector.tensor_add(
                                ps[:, :cw], ps[:, :cw], b_proj_bc[:, c0 : c0 + cw]
                            )
                        nc.vector.tensor_add(
                            z_t[:, c0 : c0 + cw], ps[:, :cw], x_sb[:, n, c0 : c0 + cw]
                        )
                    zn_t = zp.tile([P, C], BF16, tag="zn", name="zn")
                    layer_norm_tile(zp, z_t[:], zn_t[:], ln2w_bc, ln2b_bc)
                    for k in range(KC):
                        pt = tp_psum2.tile([P, P], BF16, tag="tp2", name="tp2")
                        nc.tensor.transpose(
                            pt[:], zn_t[:, k * P : (k + 1) * P], ident[:]
                        )
                        nc.any.tensor_copy(znT[:, k, n * P : (n + 1) * P], pt[:])

        # ============ scope B: MLP ============
        with ExitStack() as ctxB:
            hT_pool = ctxB.enter_context(tc.tile_pool(name="hT", bufs=1))
            hT = hT_pool.tile([P, MI, T], BF16)  # 48 KB/p

            # ---- phase 5a: fc GEMM + gelu -> hT ----
            with ExitStack() as ctx5:
                fp = ctx5.enter_context(tc.tile_pool(name="fc", bufs=3))
                fps = ctx5.enter_context(
                    tc.tile_pool(name="fc_ps", bufs=3, space="PSUM")
                )
                for m in range(MI):
                    wcol = fp.tile([P, KC, P], BF16, tag="wfc", name="wfc")
                    nc.sync.dma_start(
                        wcol[:], wfc_d[m].rearrange("p (kt j) -> p kt j", j=P)
                    )
                    for nch in range(2):
                        ps = fps.tile([P, 512], F32, tag="fc_ps", name="fc_ps")
                        for k in range(KC):
                            nc.tensor.matmul(
                                ps[:],
                                wcol[:, k, :],
                                znT[:, k, nch * 512 : (nch + 1) * 512],
                                start=(k == 0),
                                stop=(k == KC - 1),
                            )
                        bias = b_fc_pm[:, m : m + 1] if flags["b_fc"] else 0.0
                        nc.scalar.activation(
                            hT[:, m, nch * 512 : (nch + 1) * 512],
                            ps[:],
                            AF.Gelu,
                            bias=bias,
                        )

            # ---- phase 5b: out GEMM + residual -> out ----
            with ExitStack() as ctx6:
                op = ctx6.enter_context(tc.tile_pool(name="outp", bufs=3))
                ops_ = ctx6.enter_context(
                    tc.tile_pool(name="out_ps", bufs=8, space="PSUM")
                )
                for c0, cw in ((0, 512), (512, 256)):
                    psl = [
                        ops_.tile([P, 512], F32, tag="o_ps", name=f"o_ps{c0}_{n}")
                        for n in range(TT)
                    ]
                    for k in range(KI):
                        wrow = op.tile([P, 512], BF16, tag="wout", name="wout")
                        nc.sync.dma_start(wrow[:, :cw], wout_d[k, :, c0 : c0 + cw])
                        for n in range(TT):
                            nc.tensor.matmul(
                                psl[n][:, :cw],
                                hT[:, k, n * P : (n + 1) * P],
                                wrow[:, :cw],
                                start=(k == 0),
                                stop=(k == KI - 1),
                            )
                    for n in range(TT):
                        ot = op.tile([P, 512], F32, tag="ot", name="ot")
                        if flags["b_out"]:
                            nc.vector.tensor_add(
                                psl[n][:, :cw], psl[n][:, :cw], b_out_bc[:, c0 : c0 + cw]
                            )
                        nc.vector.tensor_add(
                            ot[:, :cw], psl[n][:, :cw], x_sb[:, n, c0 : c0 + cw]
                        )
                        nc.sync.dma_start(
                            out_d[n * P : (n + 1) * P, c0 : c0 + cw], ot[:, :cw]
                        )

    nc.compile()
    return nc


_CACHE = {}


def _get_program(flags):
    key = tuple(sorted(flags.items()))
    if key not in _CACHE:
        _CACHE[key] = build_program(flags)
    return _CACHE[key]


def _flags_from_inputs(inputs):
    return {
        "ln1_wb": not (
            np.all(inputs["ln1_w"] == 1.0) and np.all(inputs["ln1_b"] == 0.0)
        ),
        "ln2_wb": not (
            np.all(inputs["ln2_w"] == 1.0) and np.all(inputs["ln2_b"] == 0.0)
        ),
        "b_attn": bool(np.any(inputs["b_attn"] != 0.0)),
        "b_proj": bool(np.any(inputs["b_proj"] != 0.0)),
        "b_fc": bool(np.any(inputs["b_fc"] != 0.0)),
        "b_out": bool(np.any(inputs["b_out"] != 0.0)),
    }


def _prep_shared(inputs, flags):
    """Host-side weight cast to bf16 + re-layout for dense per-partition DMA."""
    import ml_dtypes

    bf16 = ml_dtypes.bfloat16
    w_attn = np.asarray(inputs["w_attn"], np.float32)
    w_proj = np.asarray(inputs["w_proj"], np.float32)
    w_fc = np.asarray(inputs["w_fc"], np.float32)
    w_out = np.asarray(inputs["w_out"], np.float32)

    # wqk[m] = w_attn[:, m*128:(m+1)*128] as [p, kt*128+j]
    wqk = np.ascontiguousarray(
        w_attn[:, : 2 * C].reshape(KC, P, 12, P).transpose(2, 1, 0, 3).reshape(12, P, KC * P)
    ).astype(bf16)
    wv = np.ascontiguousarray(
        w_attn[:, 2 * C :].reshape(KC, P, C).transpose(1, 0, 2).reshape(P, KC * C)
    ).astype(bf16)
    wproj = np.ascontiguousarray(
        w_proj.reshape(KC, P, C).transpose(1, 0, 2).reshape(P, KC * C)
    ).astype(bf16)
    wfc = np.ascontiguousarray(
        w_fc.reshape(KC, P, MI, P).transpose(2, 1, 0, 3).reshape(MI, P, KC * P)
    ).astype(bf16)
    wout = np.ascontiguousarray(w_out.reshape(KI, P, C)).astype(bf16)

    shared = {"wqk": wqk, "wv": wv, "wproj": wproj, "wfc": wfc, "wout": wout}
    if flags["ln1_wb"]:
        shared["ln1_w"] = np.asarray(inputs["ln1_w"], np.float32)
        shared["ln1_b"] = np.asarray(inputs["ln1_b"], np.float32)
    if flags["ln2_wb"]:
        shared["ln2_w"] = np.asarray(inputs["ln2_w"], np.float32)
        shared["ln2_b"] = np.asarray(inputs["ln2_b"], np.float32)
    for f in ["b_attn", "b_proj", "b_fc", "b_out"]:
        if flags[f]:
            shared[f] = np.asarray(inputs[f], np.float32)
    return shared


def run(inputs, trace=False):
    from concourse.bass_utils import run_bass_kernel_spmd

    inputs = {k: np.asarray(v) for k, v in inputs.items()}
    flags = _flags_from_inputs(inputs)
    nc = _get_program(flags)
    shared = _prep_shared(inputs, flags)

    B = inputs["x"].shape[0]
    core_ids = list(range(B))
    in_maps = [
        {"x": np.ascontiguousarray(inputs["x"][i], dtype=np.float32), **shared}
        for i in range(B)
    ]
    res = run_bass_kernel_spmd(nc, in_maps, core_ids, trace=trace)
    out = np.stack([res.results[i]["out"] for i in range(B)]).astype(np.float32)
    return out, res


def kernel(**inputs) -> np.ndarray:
    out, _ = run(inputs, trace=False)
    return out
